# revision 33
# baseline (speedup 1.0000x reference)
"""MoE fusion kernel for Trainium2, data-parallel across 8 NeuronCores.

Reference computation (per row b of B=16384):
    x      = concat(z_s, z_e)                    # [1024]
    wgt    = softmax(x @ rw + rb)                # [8]
    h_e    = gelu(x @ w1[e] + b1[e])             # [8, 1024]
    y_e    = h_e @ w2[e] + b2[e]                 # [8, 1024]
    ln_e   = (y_e - mu_e) * rsqrt(var_e + eps) * gamma[e] + beta[e]
    z      = sum_e wgt[e] * ln_e                 # [1024]

Sharding: batch split 8 ways (2048 rows/core), params replicated. No
collectives.

Design notes (v2):
- All matmul operands are bf16 (host-converted); PSUM accumulates fp32.
  bf16 runs at the same 1 cyc/row PE rate as fp32r but halves DMA and
  SBUF, enabling a single global supertile (weights stream exactly once)
  and deep weight prefetch.
- x reaches SBUF feature-major via DMA crossbar transposes
  (dma_start_transpose), eliminating the PE transpose + scalar copy
  pipeline entirely.
- One expert loop over the whole 2048-row block. Layer 1 keeps
  activations feature-major (hT), layer 2 uses hT chunks as stationary so
  y lands batch-major for the LayerNorm row reduction (bn_stats/bn_aggr).
- The LN sqrt is batched per 8-row-chunk group so the ACT engine swaps
  activation tables (gelu <-> sqrt) only ~4x per expert instead of per
  chunk; exp (router softmax) runs once up front.
- The softmax-weighted accumulation z += alpha*y (alpha = wgt*rstd) runs
  on the otherwise-idle GPSIMD engine via scalar_tensor_tensor; the
  -mu*alpha correction is a per-row scalar accumulated separately and
  added once at the end.
- z is DMA'd out per 128-row chunk as soon as the last expert finishes
  it, overlapping the output writeback with remaining compute.
"""
import numpy as np
from contextlib import ExitStack

import ml_dtypes

import concourse.bass as bass
import concourse.bacc as bacc
import concourse.mybir as mybir
import concourse.tile as tile
from concourse.bass_utils import run_bass_kernel_spmd

P = 128          # partitions
D = 1024         # IN_DIM == OUT_DIM
E = 8            # experts
NK = D // P      # 8 contraction chunks of 128
NCORES = 8
B_FULL = 16384
BL = B_FULL // NCORES   # 2048 rows per core
SEQ = 512               # z_s/z_e width

F32 = mybir.dt.float32
BF16 = mybir.dt.bfloat16
AF = mybir.ActivationFunctionType
ALU = mybir.AluOpType
BF16_NP = ml_dtypes.bfloat16


def _build(bl, affine):
    """Build the per-core Bass program. bl: rows per core (mult of 512).
    affine: include the gamma/beta per-expert affine (general) path."""
    nt = bl // 512          # 512-row moving tiles
    nb = bl // P            # 128-row chunks
    # groups of up to 8 chunks share one batched sqrt
    groups = [list(range(g, min(g + 8, nb))) for g in range(0, nb, 8)]

    nc = bacc.Bacc(None, target_bir_lowering=False)
    zs_d = nc.declare_dram_parameter("zs", [bl, SEQ], BF16, isOutput=False)
    ze_d = nc.declare_dram_parameter("ze", [bl, SEQ], BF16, isOutput=False)
    rw_d = nc.declare_dram_parameter("rw", [D, E], BF16, isOutput=False)
    rb_d = nc.declare_dram_parameter("rb", [E], BF16, isOutput=False)
    w1_d = nc.declare_dram_parameter("w1", [E, D, D], BF16, isOutput=False)
    b1_d = nc.declare_dram_parameter("b1", [E, D], F32, isOutput=False)
    w2_d = nc.declare_dram_parameter("w2", [E, D, D], BF16, isOutput=False)
    b2_d = nc.declare_dram_parameter("b2", [E, D], BF16, isOutput=False)
    if affine:
        gam_d = nc.declare_dram_parameter("gam", [E, D], BF16, isOutput=False)
        bet_d = nc.declare_dram_parameter("bet", [E, D], BF16, isOutput=False)
    z_d = nc.declare_dram_parameter("z", [bl, D], F32, isOutput=True)

    with tile.TileContext(nc) as tc, ExitStack() as ctx:
        consts = ctx.enter_context(tc.tile_pool(name="consts", bufs=1))
        xp = ctx.enter_context(tc.tile_pool(name="xp", bufs=1))
        wp = ctx.enter_context(tc.tile_pool(name="wp", bufs=2))
        hp = ctx.enter_context(tc.tile_pool(name="hp", bufs=9))
        yp = ctx.enter_context(tc.tile_pool(name="yp", bufs=20))
        zp = ctx.enter_context(tc.tile_pool(name="zp", bufs=nb))
        sp = ctx.enter_context(tc.tile_pool(name="sp", bufs=3))
        psH = ctx.enter_context(tc.tile_pool(name="psH", bufs=4, space="PSUM"))
        psY = ctx.enter_context(tc.tile_pool(name="psY", bufs=4, space="PSUM"))

        eps_t = consts.tile([P, 1], F32)
        nc.vector.memset(eps_t, 1e-5)
        ones_t = consts.tile([1, P], BF16)
        nc.vector.memset(ones_t, 1.0)
        rw_sb = consts.tile([P, NK, E], BF16)
        rb_sb = consts.tile([1, E], BF16)
        wsm = consts.tile([P, nb, E], F32)

        # x, feature-major: xzs[p, c, b] = zs[b, c*128+p]; ze likewise
        xzs = xp.tile([P, SEQ // P, bl], BF16, tag="xzs")
        xze = xp.tile([P, SEQ // P, bl], BF16, tag="xze")

        def xchunk(c):
            return xzs[:, c, :] if c < SEQ // P else xze[:, c - SEQ // P, :]

        def xtd(t):
            sl = slice(t * 512, (t + 1) * 512)
            nc.sync.dma_start_transpose(out=xzs[:, :, sl], in_=zs_d[sl, :])
            nc.sync.dma_start_transpose(out=xze[:, :, sl], in_=ze_d[sl, :])

        def load_weights(e):
            w1t = [wp.tile([P, D], BF16, tag="w1", bufs=16, name=f"w1_{e}_{c}")
                   for c in range(NK)]
            for c in range(NK):
                nc.sync.dma_start(out=w1t[c], in_=w1_d[e, c * P:(c + 1) * P, :])
            b1_sb = wp.tile([P, NK], F32, tag="b1", bufs=2, name=f"b1_{e}")
            nc.sync.dma_start(out=b1_sb, in_=b1_d[e].rearrange("(m p) -> p m", p=P))
            return w1t, b1_sb

        def load_weights2(e):
            w2t = [wp.tile([P, D], BF16, tag="w2", bufs=12, name=f"w2_{e}_{c}")
                   for c in range(NK)]
            b2_sb = wp.tile([P, D], BF16, tag="b2", bufs=2, name=f"b2_{e}")
            nc.sync.dma_start(out=b2_sb, in_=b2_d[e].partition_broadcast(P))
            for c in range(NK):
                nc.sync.dma_start(out=w2t[c], in_=w2_d[e, c * P:(c + 1) * P, :])
            ab = None
            if affine:
                gam_sb = wp.tile([P, D], BF16, tag="gam", bufs=2, name=f"g_{e}")
                nc.sync.dma_start(out=gam_sb, in_=gam_d[e].partition_broadcast(P))
                bet_sb = wp.tile([P, D], BF16, tag="bet", bufs=2, name=f"bt_{e}")
                nc.sync.dma_start(out=bet_sb, in_=bet_d[e].partition_broadcast(P))
                ab = (gam_sb, bet_sb)
            return w2t, b2_sb, ab

        def router(bb):
            ps_r = psY.tile([P, E], F32, tag="y", name=f"psr_{bb}")
            for ci, c in enumerate(list(range(SEQ // P)) + list(range(SEQ // P, NK))):
                nc.tensor.matmul(ps_r, xchunk(c)[:, bb * P:(bb + 1) * P],
                                 rw_sb[:, c, :], start=(ci == 0), stop=False)
            nc.tensor.matmul(ps_r, ones_t, rb_sb, start=False, stop=True)
            ex = sp.tile([P, E], F32, tag="ex", bufs=2, name=f"ex_{bb}")
            nc.scalar.activation(out=ex, in_=ps_r, func=AF.Exp)
            sm = sp.tile([P, 1], F32, tag="sm", bufs=2, name=f"sm_{bb}")
            nc.vector.tensor_reduce(out=sm, in_=ex, axis=mybir.AxisListType.X,
                                    op=ALU.add)
            rc = sp.tile([P, 1], F32, tag="rc", bufs=2, name=f"rc_{bb}")
            nc.vector.reciprocal(out=rc, in_=sm)
            nc.vector.tensor_scalar_mul(out=wsm[:, bb, :], in0=ex, scalar1=rc)

        z_t = [zp.tile([P, D], F32, tag="z", name=f"z_{bb}") for bb in range(nb)]
        if affine:
            for bb in range(nb):
                nc.gpsimd.memset(z_t[bb], 0.0)

        # ---- startup ----
        # DMA order is the startup critical path: the first quarter of x-t0
        # and w1-e0's first chunks come first so layer-1 can begin while the
        # rest streams; router consts / w2-e0 / x-t1 are consumed later.
        xtd(0)
        w1t, b1_sb = load_weights(0)
        nc.sync.dma_start(out=rw_sb, in_=rw_d[:].rearrange("(c p) e -> p c e", p=P))
        nc.sync.dma_start(out=rb_sb, in_=rb_d[:].rearrange("(one e) -> one e", one=1))
        w2t, b2_sb, ab = load_weights2(0)
        if nt > 1:
            xtd(1)

        # pull the PE p-state ramp start forward while the startup DMAs
        # stream (the ramp window is wall-clock from the first PE op)
        for i in range(0):
            wu = psY.tile([P, P], F32, tag="y", name=f"wu_{i}")
            nc.tensor.matmul(wu, ones_t, ones_t, start=True, stop=True)

        yb_all = {}     # bb -> [yb_n0, yb_n1]
        mv_g = {}       # group index -> mv tile

        def layer1(e, t):
            h8 = []
            split = 1
            for m in range(NK):
                hc = hp.tile([P, 512], BF16, tag="h8", name=f"h_{e}_{t}_{m}")
                w = 512 // split
                for hf in range(split):
                    ps_h = psH.tile([P, w], F32, tag="h",
                                    name=f"ph_{e}_{t}_{m}_{hf}")
                    lo = t * 512 + hf * w
                    for ci, c in enumerate(list(range(SEQ // P))
                                           + list(range(SEQ // P, NK))):
                        nc.tensor.matmul(ps_h, w1t[c][:, m * P:(m + 1) * P],
                                         xchunk(c)[:, lo:lo + w],
                                         start=(ci == 0), stop=(ci == NK - 1))
                    nc.scalar.activation(out=hc[:, hf * w:(hf + 1) * w],
                                         in_=ps_h, func=AF.Gelu,
                                         bias=b1_sb[:, m:m + 1], scale=1.0)
                h8.append(hc)
            return h8

        def layer2(e, t, h8):
            for s in range(4):
                bb = t * 4 + s
                gi, slot = bb // 8, bb % 8
                if slot == 0:
                    mv_g[gi] = sp.tile([P, 2, 8], F32, tag="mv", name=f"mv_{e}_{gi}")
                ps_ys = [psY.tile([P, 512], F32, tag="y", name=f"py_{e}_{bb}_{n}")
                         for n in range(2)]
                for c in range(NK):
                    for n in range(2):
                        nc.tensor.matmul(ps_ys[n], h8[c][:, s * P:(s + 1) * P],
                                         w2t[c][:, n * 512:(n + 1) * 512],
                                         start=(c == 0), stop=(c == NK - 1))
                ybs = []
                for n in range(2):
                    yb = yp.tile([P, 512], BF16, tag="yb",
                                 name=f"yb_{e}_{bb}_{n}")
                    nc.vector.tensor_add(yb, ps_ys[n],
                                         b2_sb[:, n * 512:(n + 1) * 512])
                    ybs.append(yb)
                yb_all[bb] = ybs
                stats = sp.tile([P, 2, 6], F32, tag="st", bufs=4,
                                name=f"st_{e}_{bb}")
                nc.vector.bn_stats(out=stats[:, 0, :], in_=ybs[0])
                nc.vector.bn_stats(out=stats[:, 1, :], in_=ybs[1])
                nc.vector.bn_aggr(out=mv_g[gi][:, :, slot], in_=stats)

        def bb_finish(e, bb, tail):
            """Per-row-chunk LN scale + weighted z accumulation, engine-split:
            ct = alpha*y - mu*alpha via ACT (n0 half, scale+bias Identity) and
            DVE (n1 half, dual-scalar tensor_scalar); Pool adds into z."""
            gi, slot = bb // 8, bb % 8
            mv = mv_g[gi]
            sd = sp.tile([P, 1], F32, tag="sd", name=f"sd_{e}_{bb}")
            nc.scalar.activation(out=sd, in_=mv[:, 1, slot:slot + 1],
                                 func=AF.Sqrt, bias=eps_t, scale=1.0)
            rs = sp.tile([P, 1], F32, tag="rs", name=f"rs_{e}_{bb}")
            nc.vector.reciprocal(out=rs, in_=sd)
            al = sp.tile([P, 1], F32, tag="al", name=f"al_{e}_{bb}")
            nc.vector.tensor_mul(al, rs, wsm[:, bb, e:e + 1])
            nb1 = sp.tile([P, 1], F32, tag="nb", name=f"nb_{e}_{bb}")
            nc.vector.scalar_tensor_tensor(
                out=nb1, in0=mv[:, 0, slot:slot + 1], scalar=-1.0,
                in1=al, op0=ALU.mult, op1=ALU.mult)
            for n in range(2):
                zsl = z_t[bb][:, n * 512:(n + 1) * 512]
                yb = yb_all[bb][n]
                direct = not affine and e == 0
                dst = zsl if direct else yp.tile([P, 512], F32, tag="ct",
                                                 bufs=4, name=f"ct_{e}_{bb}_{n}")
                if n == 0:
                    nc.scalar.activation(out=dst, in_=yb, func=AF.Identity,
                                         bias=nb1, scale=al)
                else:
                    nc.vector.tensor_scalar(out=dst, in0=yb, scalar1=al,
                                            scalar2=nb1, op0=ALU.mult,
                                            op1=ALU.add)
                if direct:
                    continue
                if affine:
                    gam_sb, bet_sb = ab
                    nc.gpsimd.tensor_mul(dst, dst,
                                         gam_sb[:, n * 512:(n + 1) * 512])
                    bw = yp.tile([P, 512], F32, tag="bw", bufs=3,
                                 name=f"bw_{e}_{bb}_{n}")
                    nc.vector.tensor_scalar_mul(
                        out=bw, in0=bet_sb[:, n * 512:(n + 1) * 512],
                        scalar1=wsm[:, bb, e:e + 1])
                    nc.gpsimd.tensor_add(dst, dst, bw)
                eng = nc.vector if (tail and n == 1) else nc.gpsimd
                eng.tensor_add(zsl, zsl, dst)
            if tail:
                nc.sync.dma_start(out=z_d[bb * P:(bb + 1) * P, :], in_=z_t[bb])

        # ---- expert loop ----
        for e in range(E):
            for t in range(nt):
                h8 = layer1(e, t)
                if e == 0:
                    # Router chunks for t ride right after layer1(t) so the
                    # xt slice they need is already resident; late x
                    # transposes and next-expert weights are issued here so
                    # the startup DMA window holds only x-t0 + w1/w2-e0.
                    for bb in range(4 * t, min(4 * (t + 1), nb)):
                        router(bb)
                    if t + 2 < nt:
                        xtd(t + 2)
                    if t == 0:
                        nxt_w = load_weights(1) if E > 1 else None
                layer2(e, t, h8)
                for bb in range(4 * t, min(4 * (t + 1), nb)):
                    bb_finish(e, bb, tail=(e == E - 1))
            if e + 1 < E:
                if e > 0:
                    nxt_w = load_weights(e + 1)
                w1t, b1_sb = nxt_w
                w2t, b2_sb, ab = load_weights2(e + 1)

    nc.compile()
    return nc


_NC_CACHE = {}
_RUNNER_CACHE = {}


def _pjrt_runner(nc):
    """Reusable jitted PJRT executable for `nc` (axon path). Mirrors
    bass2jax.run_bass_via_pjrt but is cached so repeated kernel() calls do
    not re-trace/recompile."""
    import jax
    from jax.sharding import Mesh, PartitionSpec
    from jax.experimental.shard_map import shard_map
    from concourse.bass2jax import (_bass_exec_p, install_neuronx_cc_hook,
                                    partition_id_tensor)

    install_neuronx_cc_hook()
    partition_name = nc.partition_id_tensor.name if nc.partition_id_tensor else None
    in_names, out_names, out_avals = [], [], []
    for alloc in nc.m.functions[0].allocations:
        if not isinstance(alloc, mybir.MemoryLocationSet):
            continue
        name = alloc.memorylocations[0].name
        if alloc.kind == "ExternalInput":
            if name != partition_name:
                in_names.append(name)
        elif alloc.kind == "ExternalOutput":
            out_names.append(name)
            out_avals.append(jax.core.ShapedArray(tuple(alloc.tensor_shape),
                                                  mybir.dt.np(alloc.dtype)))
    n_params = len(in_names)
    all_in = list(in_names) + list(out_names)
    if partition_name is not None:
        all_in.append(partition_name)

    def _body(*args):
        operands = list(args)
        if partition_name is not None:
            operands.append(partition_id_tensor())
        return tuple(_bass_exec_p.bind(
            *operands, out_avals=tuple(out_avals), in_names=tuple(all_in),
            out_names=tuple(out_names), lowering_input_output_aliases=(),
            sim_require_finite=True, sim_require_nnan=True, nc=nc))

    devices = jax.devices()[:NCORES]
    assert len(devices) == NCORES
    mesh = Mesh(np.asarray(devices), ("core",))
    specs = (PartitionSpec("core"),) * (n_params + len(out_names))
    fn = jax.jit(shard_map(_body, mesh=mesh, in_specs=specs,
                           out_specs=(PartitionSpec("core"),) * len(out_names),
                           check_rep=False), keep_unused=True)
    return fn, in_names, out_names, out_avals


def _run_cached(nc, in_maps):
    """Run via cached jitted executable with retry; fall back to
    run_bass_kernel_spmd. Retries cover transient device wedges
    (NRT_EXEC_UNIT_UNRECOVERABLE) seen after rapid process turnover."""
    import time as _time
    last_exc = None
    for attempt in range(3):
        try:
            return _run_once(nc, in_maps)
        except Exception as e:
            last_exc = e
            _RUNNER_CACHE.pop(id(nc), None)
            _time.sleep(10 * (attempt + 1))
    raise last_exc


def _run_once(nc, in_maps):
    import jax
    try:
        from concourse._compat import axon_active
        if not axon_active():
            raise RuntimeError("not axon; use native path")
        key = id(nc)
        if key not in _RUNNER_CACHE:
            _RUNNER_CACHE[key] = _pjrt_runner(nc)
        fn, in_names, out_names, out_avals = _RUNNER_CACHE[key]
        concat_in = [np.concatenate([np.asarray(in_maps[c][k])
                                     for c in range(NCORES)], axis=0)
                     for k in in_names]
        concat_zeros = [np.zeros((NCORES * a.shape[0], *a.shape[1:]), a.dtype)
                        for a in out_avals]
        outs = fn(*concat_in, *concat_zeros)
        jax.block_until_ready(outs)
        out_np = [np.asarray(o) for o in outs]
        return [{name: out_np[i].reshape(NCORES, *out_avals[i].shape)[c]
                 for i, name in enumerate(out_names)}
                for c in range(NCORES)]
    except Exception:
        res = run_bass_kernel_spmd(nc, in_maps, core_ids=list(range(NCORES)))
        return res.results


def _get_nc(bl, affine):
    key = (bl, affine)
    if key not in _NC_CACHE:
        _NC_CACHE[key] = _build(bl, affine)
    return _NC_CACHE[key]


def make_in_maps(z_s, z_e, router_w, router_b, w1, b1, w2, b2, gamma, beta,
                 affine, bl):
    """Host-side input prep: bf16 conversion + per-core batch sharding."""
    bf = lambda a: np.asarray(a, dtype=np.float32).astype(BF16_NP)
    zs8, ze8 = bf(z_s), bf(z_e)
    rw8, rb8 = bf(router_w), bf(router_b)
    w18, w28, b28 = bf(w1), bf(w2), bf(b2)
    b1f = np.ascontiguousarray(np.asarray(b1, dtype=np.float32))
    in_maps = []
    for c in range(NCORES):
        sl = slice(c * bl, (c + 1) * bl)
        m = {
            "zs": zs8[sl], "ze": ze8[sl],
            "rw": rw8, "rb": rb8,
            "w1": w18, "b1": b1f, "w2": w28, "b2": b28,
        }
        if affine:
            m["gam"] = bf(gamma)
            m["bet"] = bf(beta)
        in_maps.append(m)
    return in_maps


def kernel(z_s, z_e, router_w, router_b, w1, b1, w2, b2, gamma, beta):
    gamma = np.asarray(gamma, dtype=np.float32)
    beta = np.asarray(beta, dtype=np.float32)
    b_full = np.asarray(z_s).shape[0]
    assert b_full % NCORES == 0, f"batch {b_full} not divisible by {NCORES} cores"
    bl = b_full // NCORES
    assert bl % 512 == 0, f"per-core batch {bl} must be a multiple of 512"

    affine = not (np.all(gamma == 1.0) and np.all(beta == 0.0))
    nc = _get_nc(bl, affine)
    in_maps = make_in_maps(z_s, z_e, router_w, router_b, w1, b1, w2, b2,
                           gamma, beta, affine, bl)
    results = _run_cached(nc, in_maps)
    return np.concatenate([results[c]["z"] for c in range(NCORES)], axis=0)


# revision 42
# speedup vs baseline: 1.0008x; 1.0008x over previous
"""MoE fusion kernel for Trainium2, data-parallel across 8 NeuronCores.

Reference computation (per row b of B=16384):
    x      = concat(z_s, z_e)                    # [1024]
    wgt    = softmax(x @ rw + rb)                # [8]
    h_e    = gelu(x @ w1[e] + b1[e])             # [8, 1024]
    y_e    = h_e @ w2[e] + b2[e]                 # [8, 1024]
    ln_e   = (y_e - mu_e) * rsqrt(var_e + eps) * gamma[e] + beta[e]
    z      = sum_e wgt[e] * ln_e                 # [1024]

Sharding: batch split 8 ways (2048 rows/core), params replicated. No
collectives.

Design notes (v2):
- All matmul operands are bf16 (host-converted); PSUM accumulates fp32.
  bf16 runs at the same 1 cyc/row PE rate as fp32r but halves DMA and
  SBUF, enabling a single global supertile (weights stream exactly once)
  and deep weight prefetch.
- x reaches SBUF feature-major via DMA crossbar transposes
  (dma_start_transpose), eliminating the PE transpose + scalar copy
  pipeline entirely.
- One expert loop over the whole 2048-row block. Layer 1 keeps
  activations feature-major (hT), layer 2 uses hT chunks as stationary so
  y lands batch-major for the LayerNorm row reduction (bn_stats/bn_aggr).
- The LN sqrt is batched per 8-row-chunk group so the ACT engine swaps
  activation tables (gelu <-> sqrt) only ~4x per expert instead of per
  chunk; exp (router softmax) runs once up front.
- The softmax-weighted accumulation z += alpha*y (alpha = wgt*rstd) runs
  on the otherwise-idle GPSIMD engine via scalar_tensor_tensor; the
  -mu*alpha correction is a per-row scalar accumulated separately and
  added once at the end.
- z is DMA'd out per 128-row chunk as soon as the last expert finishes
  it, overlapping the output writeback with remaining compute.
"""
import numpy as np
from contextlib import ExitStack

import ml_dtypes

import concourse.bass as bass
import concourse.bacc as bacc
import concourse.mybir as mybir
import concourse.tile as tile
from concourse.bass_utils import run_bass_kernel_spmd

P = 128          # partitions
D = 1024         # IN_DIM == OUT_DIM
E = 8            # experts
NK = D // P      # 8 contraction chunks of 128
NCORES = 8
B_FULL = 16384
BL = B_FULL // NCORES   # 2048 rows per core
SEQ = 512               # z_s/z_e width

F32 = mybir.dt.float32
BF16 = mybir.dt.bfloat16
AF = mybir.ActivationFunctionType
ALU = mybir.AluOpType
BF16_NP = ml_dtypes.bfloat16


def _build(bl, affine):
    """Build the per-core Bass program. bl: rows per core (mult of 512).
    affine: include the gamma/beta per-expert affine (general) path."""
    nt = bl // 512          # 512-row moving tiles
    nb = bl // P            # 128-row chunks
    # groups of up to 8 chunks share one batched sqrt
    groups = [list(range(g, min(g + 8, nb))) for g in range(0, nb, 8)]

    nc = bacc.Bacc(None, target_bir_lowering=False)
    zs_d = nc.declare_dram_parameter("zs", [bl, SEQ], BF16, isOutput=False)
    ze_d = nc.declare_dram_parameter("ze", [bl, SEQ], BF16, isOutput=False)
    rw_d = nc.declare_dram_parameter("rw", [D, E], BF16, isOutput=False)
    rb_d = nc.declare_dram_parameter("rb", [E], BF16, isOutput=False)
    w1_d = nc.declare_dram_parameter("w1", [E, D, D], BF16, isOutput=False)
    b1_d = nc.declare_dram_parameter("b1", [E, D], F32, isOutput=False)
    w2_d = nc.declare_dram_parameter("w2", [E, D, D], BF16, isOutput=False)
    b2_d = nc.declare_dram_parameter("b2", [E, D], BF16, isOutput=False)
    if affine:
        gam_d = nc.declare_dram_parameter("gam", [E, D], BF16, isOutput=False)
        bet_d = nc.declare_dram_parameter("bet", [E, D], BF16, isOutput=False)
    z_d = nc.declare_dram_parameter("z", [bl, D], F32, isOutput=True)

    with tile.TileContext(nc) as tc, ExitStack() as ctx:
        consts = ctx.enter_context(tc.tile_pool(name="consts", bufs=1))
        xp = ctx.enter_context(tc.tile_pool(name="xp", bufs=1))
        wp = ctx.enter_context(tc.tile_pool(name="wp", bufs=2))
        hp = ctx.enter_context(tc.tile_pool(name="hp", bufs=9))
        yp = ctx.enter_context(tc.tile_pool(name="yp", bufs=20))
        zp = ctx.enter_context(tc.tile_pool(name="zp", bufs=nb))
        sp = ctx.enter_context(tc.tile_pool(name="sp", bufs=3))
        psH = ctx.enter_context(tc.tile_pool(name="psH", bufs=4, space="PSUM"))
        psY = ctx.enter_context(tc.tile_pool(name="psY", bufs=4, space="PSUM"))

        eps_t = consts.tile([P, 1], F32)
        nc.vector.memset(eps_t, 1e-5)
        ones_t = consts.tile([1, P], BF16)
        nc.vector.memset(ones_t, 1.0)
        rw_sb = consts.tile([P, NK, E], BF16)
        rb_sb = consts.tile([1, E], BF16)
        wsm = consts.tile([P, nb, E], F32)

        # x, feature-major: xzs[p, c, b] = zs[b, c*128+p]; ze likewise
        xzs = xp.tile([P, SEQ // P, bl], BF16, tag="xzs")
        xze = xp.tile([P, SEQ // P, bl], BF16, tag="xze")

        def xchunk(c):
            return xzs[:, c, :] if c < SEQ // P else xze[:, c - SEQ // P, :]

        def xtd(t):
            sl = slice(t * 512, (t + 1) * 512)
            nc.sync.dma_start_transpose(out=xzs[:, :, sl], in_=zs_d[sl, :])
            nc.sync.dma_start_transpose(out=xze[:, :, sl], in_=ze_d[sl, :])

        def load_weights(e):
            w1t = [wp.tile([P, D], BF16, tag="w1", bufs=16, name=f"w1_{e}_{c}")
                   for c in range(NK)]
            for c in range(NK):
                nc.sync.dma_start(out=w1t[c], in_=w1_d[e, c * P:(c + 1) * P, :])
            b1_sb = wp.tile([P, NK], F32, tag="b1", bufs=2, name=f"b1_{e}")
            nc.sync.dma_start(out=b1_sb, in_=b1_d[e].rearrange("(m p) -> p m", p=P))
            return w1t, b1_sb

        def load_weights2(e):
            w2t = [wp.tile([P, D], BF16, tag="w2", bufs=12, name=f"w2_{e}_{c}")
                   for c in range(NK)]
            b2_sb = wp.tile([P, D], BF16, tag="b2", bufs=2, name=f"b2_{e}")
            nc.sync.dma_start(out=b2_sb, in_=b2_d[e].partition_broadcast(P))
            for c in range(NK):
                nc.sync.dma_start(out=w2t[c], in_=w2_d[e, c * P:(c + 1) * P, :])
            ab = None
            if affine:
                gam_sb = wp.tile([P, D], BF16, tag="gam", bufs=2, name=f"g_{e}")
                nc.sync.dma_start(out=gam_sb, in_=gam_d[e].partition_broadcast(P))
                bet_sb = wp.tile([P, D], BF16, tag="bet", bufs=2, name=f"bt_{e}")
                nc.sync.dma_start(out=bet_sb, in_=bet_d[e].partition_broadcast(P))
                ab = (gam_sb, bet_sb)
            return w2t, b2_sb, ab

        def router(bb):
            ps_r = psY.tile([P, E], F32, tag="y", name=f"psr_{bb}")
            for ci, c in enumerate(list(range(SEQ // P)) + list(range(SEQ // P, NK))):
                nc.tensor.matmul(ps_r, xchunk(c)[:, bb * P:(bb + 1) * P],
                                 rw_sb[:, c, :], start=(ci == 0), stop=False)
            nc.tensor.matmul(ps_r, ones_t, rb_sb, start=False, stop=True)
            ex = sp.tile([P, E], F32, tag="ex", bufs=2, name=f"ex_{bb}")
            nc.scalar.activation(out=ex, in_=ps_r, func=AF.Exp)
            sm = sp.tile([P, 1], F32, tag="sm", bufs=2, name=f"sm_{bb}")
            nc.vector.tensor_reduce(out=sm, in_=ex, axis=mybir.AxisListType.X,
                                    op=ALU.add)
            rc = sp.tile([P, 1], F32, tag="rc", bufs=2, name=f"rc_{bb}")
            nc.vector.reciprocal(out=rc, in_=sm)
            nc.vector.tensor_scalar_mul(out=wsm[:, bb, :], in0=ex, scalar1=rc)

        z_t = [zp.tile([P, D], F32, tag="z", name=f"z_{bb}") for bb in range(nb)]
        if affine:
            for bb in range(nb):
                nc.gpsimd.memset(z_t[bb], 0.0)

        # ---- startup ----
        # DMA order is the startup critical path: the first quarter of x-t0
        # and w1-e0's first chunks come first so layer-1 can begin while the
        # rest streams; router consts / w2-e0 / x-t1 are consumed later.
        xtd(0)
        w1t, b1_sb = load_weights(0)
        nc.sync.dma_start(out=rw_sb, in_=rw_d[:].rearrange("(c p) e -> p c e", p=P))
        nc.sync.dma_start(out=rb_sb, in_=rb_d[:].rearrange("(one e) -> one e", one=1))
        w2t, b2_sb, ab = load_weights2(0)
        if nt > 1:
            xtd(1)

        # pull the PE p-state ramp start forward while the startup DMAs
        # stream (the ramp window is wall-clock from the first PE op)
        for i in range(0):
            wu = psY.tile([P, P], F32, tag="y", name=f"wu_{i}")
            nc.tensor.matmul(wu, ones_t, ones_t, start=True, stop=True)

        yb_all = {}     # bb -> [yb_n0, yb_n1]
        mv_g = {}       # group index -> mv tile

        def layer1(e, t):
            h8 = []
            split = 1
            for m in range(NK):
                hc = hp.tile([P, 512], BF16, tag="h8", name=f"h_{e}_{t}_{m}")
                w = 512 // split
                for hf in range(split):
                    ps_h = psH.tile([P, w], F32, tag="h",
                                    name=f"ph_{e}_{t}_{m}_{hf}")
                    lo = t * 512 + hf * w
                    for ci, c in enumerate(list(range(SEQ // P))
                                           + list(range(SEQ // P, NK))):
                        nc.tensor.matmul(ps_h, w1t[c][:, m * P:(m + 1) * P],
                                         xchunk(c)[:, lo:lo + w],
                                         start=(ci == 0), stop=(ci == NK - 1))
                    nc.scalar.activation(out=hc[:, hf * w:(hf + 1) * w],
                                         in_=ps_h, func=AF.Gelu,
                                         bias=b1_sb[:, m:m + 1], scale=1.0)
                h8.append(hc)
            return h8

        def layer2(e, t, h8):
            for s in range(4):
                bb = t * 4 + s
                gi, slot = bb // 8, bb % 8
                if slot == 0:
                    mv_g[gi] = sp.tile([P, 2, 8], F32, tag="mv", name=f"mv_{e}_{gi}")
                ps_ys = [psY.tile([P, 512], F32, tag="y", name=f"py_{e}_{bb}_{n}")
                         for n in range(2)]
                if e == E - 1 and t == nt - 1:
                    # n-outer: the n=0 half's PSUM completes a full c-loop
                    # early, so its drain+stats overlap the n=1 matmuls and
                    # shorten the end-of-kernel chain
                    for n in range(2):
                        for c in range(NK):
                            nc.tensor.matmul(ps_ys[n],
                                             h8[c][:, s * P:(s + 1) * P],
                                             w2t[c][:, n * 512:(n + 1) * 512],
                                             start=(c == 0), stop=(c == NK - 1))
                else:
                    for c in range(NK):
                        for n in range(2):
                            nc.tensor.matmul(ps_ys[n],
                                             h8[c][:, s * P:(s + 1) * P],
                                             w2t[c][:, n * 512:(n + 1) * 512],
                                             start=(c == 0), stop=(c == NK - 1))
                ybs = []
                for n in range(2):
                    yb = yp.tile([P, 512], BF16, tag="yb",
                                 name=f"yb_{e}_{bb}_{n}")
                    nc.vector.tensor_add(yb, ps_ys[n],
                                         b2_sb[:, n * 512:(n + 1) * 512])
                    ybs.append(yb)
                yb_all[bb] = ybs
                stats = sp.tile([P, 2, 6], F32, tag="st", bufs=4,
                                name=f"st_{e}_{bb}")
                nc.vector.bn_stats(out=stats[:, 0, :], in_=ybs[0])
                nc.vector.bn_stats(out=stats[:, 1, :], in_=ybs[1])
                nc.vector.bn_aggr(out=mv_g[gi][:, :, slot], in_=stats)
                bb_finish(e, bb, tail=(e == E - 1))

        def bb_finish(e, bb, tail):
            """Per-row-chunk LN scale + weighted z accumulation, engine-split:
            ct = alpha*y - mu*alpha via ACT (n0 half, scale+bias Identity) and
            DVE (n1 half, dual-scalar tensor_scalar); Pool adds into z."""
            gi, slot = bb // 8, bb % 8
            mv = mv_g[gi]
            sd = sp.tile([P, 1], F32, tag="sd", name=f"sd_{e}_{bb}")
            nc.scalar.activation(out=sd, in_=mv[:, 1, slot:slot + 1],
                                 func=AF.Sqrt, bias=eps_t, scale=1.0)
            rs = sp.tile([P, 1], F32, tag="rs", name=f"rs_{e}_{bb}")
            nc.vector.reciprocal(out=rs, in_=sd)
            al = sp.tile([P, 1], F32, tag="al", name=f"al_{e}_{bb}")
            nc.vector.tensor_mul(al, rs, wsm[:, bb, e:e + 1])
            nb1 = sp.tile([P, 1], F32, tag="nb", name=f"nb_{e}_{bb}")
            nc.vector.scalar_tensor_tensor(
                out=nb1, in0=mv[:, 0, slot:slot + 1], scalar=-1.0,
                in1=al, op0=ALU.mult, op1=ALU.mult)
            for n in range(2):
                zsl = z_t[bb][:, n * 512:(n + 1) * 512]
                yb = yb_all[bb][n]
                direct = not affine and e == 0
                dst = zsl if direct else yp.tile([P, 512], F32, tag="ct",
                                                 bufs=4, name=f"ct_{e}_{bb}_{n}")
                if n == 0:
                    nc.scalar.activation(out=dst, in_=yb, func=AF.Identity,
                                         bias=nb1, scale=al)
                else:
                    nc.vector.tensor_scalar(out=dst, in0=yb, scalar1=al,
                                            scalar2=nb1, op0=ALU.mult,
                                            op1=ALU.add)
                if direct:
                    continue
                if affine:
                    gam_sb, bet_sb = ab
                    nc.gpsimd.tensor_mul(dst, dst,
                                         gam_sb[:, n * 512:(n + 1) * 512])
                    bw = yp.tile([P, 512], F32, tag="bw", bufs=3,
                                 name=f"bw_{e}_{bb}_{n}")
                    nc.vector.tensor_scalar_mul(
                        out=bw, in0=bet_sb[:, n * 512:(n + 1) * 512],
                        scalar1=wsm[:, bb, e:e + 1])
                    nc.gpsimd.tensor_add(dst, dst, bw)
                eng = nc.vector if (tail and n == 1) else nc.gpsimd
                eng.tensor_add(zsl, zsl, dst)
            if tail:
                nc.sync.dma_start(out=z_d[bb * P:(bb + 1) * P, :], in_=z_t[bb])

        # ---- expert loop ----
        for e in range(E):
            for t in range(nt):
                h8 = layer1(e, t)
                if e == 0:
                    # Router chunks for t ride right after layer1(t) so the
                    # xt slice they need is already resident; late x
                    # transposes and next-expert weights are issued here so
                    # the startup DMA window holds only x-t0 + w1/w2-e0.
                    for bb in range(4 * t, min(4 * (t + 1), nb)):
                        router(bb)
                    if t + 2 < nt:
                        xtd(t + 2)
                    if t == 0:
                        nxt_w = load_weights(1) if E > 1 else None
                layer2(e, t, h8)
            if e + 1 < E:
                if e > 0:
                    nxt_w = load_weights(e + 1)
                w1t, b1_sb = nxt_w
                w2t, b2_sb, ab = load_weights2(e + 1)

    nc.compile()
    return nc


_NC_CACHE = {}
_RUNNER_CACHE = {}


def _pjrt_runner(nc):
    """Reusable jitted PJRT executable for `nc` (axon path). Mirrors
    bass2jax.run_bass_via_pjrt but is cached so repeated kernel() calls do
    not re-trace/recompile."""
    import jax
    from jax.sharding import Mesh, PartitionSpec
    from jax.experimental.shard_map import shard_map
    from concourse.bass2jax import (_bass_exec_p, install_neuronx_cc_hook,
                                    partition_id_tensor)

    install_neuronx_cc_hook()
    partition_name = nc.partition_id_tensor.name if nc.partition_id_tensor else None
    in_names, out_names, out_avals = [], [], []
    for alloc in nc.m.functions[0].allocations:
        if not isinstance(alloc, mybir.MemoryLocationSet):
            continue
        name = alloc.memorylocations[0].name
        if alloc.kind == "ExternalInput":
            if name != partition_name:
                in_names.append(name)
        elif alloc.kind == "ExternalOutput":
            out_names.append(name)
            out_avals.append(jax.core.ShapedArray(tuple(alloc.tensor_shape),
                                                  mybir.dt.np(alloc.dtype)))
    n_params = len(in_names)
    all_in = list(in_names) + list(out_names)
    if partition_name is not None:
        all_in.append(partition_name)

    def _body(*args):
        operands = list(args)
        if partition_name is not None:
            operands.append(partition_id_tensor())
        return tuple(_bass_exec_p.bind(
            *operands, out_avals=tuple(out_avals), in_names=tuple(all_in),
            out_names=tuple(out_names), lowering_input_output_aliases=(),
            sim_require_finite=True, sim_require_nnan=True, nc=nc))

    devices = jax.devices()[:NCORES]
    assert len(devices) == NCORES
    mesh = Mesh(np.asarray(devices), ("core",))
    specs = (PartitionSpec("core"),) * (n_params + len(out_names))
    fn = jax.jit(shard_map(_body, mesh=mesh, in_specs=specs,
                           out_specs=(PartitionSpec("core"),) * len(out_names),
                           check_rep=False), keep_unused=True)
    return fn, in_names, out_names, out_avals


def _run_cached(nc, in_maps):
    """Run via cached jitted executable with retry; fall back to
    run_bass_kernel_spmd. Retries cover transient device wedges
    (NRT_EXEC_UNIT_UNRECOVERABLE) seen after rapid process turnover."""
    import time as _time
    last_exc = None
    for attempt in range(3):
        try:
            return _run_once(nc, in_maps)
        except Exception as e:
            last_exc = e
            _RUNNER_CACHE.pop(id(nc), None)
            _time.sleep(10 * (attempt + 1))
    raise last_exc


def _run_once(nc, in_maps):
    import jax
    try:
        from concourse._compat import axon_active
        if not axon_active():
            raise RuntimeError("not axon; use native path")
        key = id(nc)
        if key not in _RUNNER_CACHE:
            _RUNNER_CACHE[key] = _pjrt_runner(nc)
        fn, in_names, out_names, out_avals = _RUNNER_CACHE[key]
        concat_in = [np.concatenate([np.asarray(in_maps[c][k])
                                     for c in range(NCORES)], axis=0)
                     for k in in_names]
        concat_zeros = [np.zeros((NCORES * a.shape[0], *a.shape[1:]), a.dtype)
                        for a in out_avals]
        outs = fn(*concat_in, *concat_zeros)
        jax.block_until_ready(outs)
        out_np = [np.asarray(o) for o in outs]
        return [{name: out_np[i].reshape(NCORES, *out_avals[i].shape)[c]
                 for i, name in enumerate(out_names)}
                for c in range(NCORES)]
    except Exception:
        res = run_bass_kernel_spmd(nc, in_maps, core_ids=list(range(NCORES)))
        return res.results


def _get_nc(bl, affine):
    key = (bl, affine)
    if key not in _NC_CACHE:
        _NC_CACHE[key] = _build(bl, affine)
    return _NC_CACHE[key]


def make_in_maps(z_s, z_e, router_w, router_b, w1, b1, w2, b2, gamma, beta,
                 affine, bl):
    """Host-side input prep: bf16 conversion + per-core batch sharding."""
    bf = lambda a: np.asarray(a, dtype=np.float32).astype(BF16_NP)
    zs8, ze8 = bf(z_s), bf(z_e)
    rw8, rb8 = bf(router_w), bf(router_b)
    w18, w28, b28 = bf(w1), bf(w2), bf(b2)
    b1f = np.ascontiguousarray(np.asarray(b1, dtype=np.float32))
    in_maps = []
    for c in range(NCORES):
        sl = slice(c * bl, (c + 1) * bl)
        m = {
            "zs": zs8[sl], "ze": ze8[sl],
            "rw": rw8, "rb": rb8,
            "w1": w18, "b1": b1f, "w2": w28, "b2": b28,
        }
        if affine:
            m["gam"] = bf(gamma)
            m["bet"] = bf(beta)
        in_maps.append(m)
    return in_maps


def kernel(z_s, z_e, router_w, router_b, w1, b1, w2, b2, gamma, beta):
    gamma = np.asarray(gamma, dtype=np.float32)
    beta = np.asarray(beta, dtype=np.float32)
    b_full = np.asarray(z_s).shape[0]
    assert b_full % NCORES == 0, f"batch {b_full} not divisible by {NCORES} cores"
    bl = b_full // NCORES
    assert bl % 512 == 0, f"per-core batch {bl} must be a multiple of 512"

    affine = not (np.all(gamma == 1.0) and np.all(beta == 0.0))
    nc = _get_nc(bl, affine)
    in_maps = make_in_maps(z_s, z_e, router_w, router_b, w1, b1, w2, b2,
                           gamma, beta, affine, bl)
    results = _run_cached(nc, in_maps)
    return np.concatenate([results[c]["z"] for c in range(NCORES)], axis=0)
